# revision 3
# baseline (speedup 1.0000x reference)
"""Trainium2 Bass kernel: GPT2 block with ctx-pred sparse attention.

Sharding: tokens (blocks of 17) across 8 cores; window-head tokens
replicated. Each core runs the full block on its 272-token slice per
batch with zero collectives. All activations in transposed layout
[feature(partition), token(free)]; matmuls in bf16 with fp32 accum.

Attention computes transposed scores S^T = K Q^T directly (no PE
transposes), does softmax across partitions via ones-matmul column
sums, and broadcasts the per-token reciprocal with a K=1 PE matmul.
PSUM pools are phase-scoped so the attention loop gets 6 banks of
double-buffering.
"""
import numpy as np
import ml_dtypes

import concourse.bass as bass
import concourse.mybir as mybir
import concourse.tile as tile
from concourse import bacc
from concourse.bass_utils import run_bass_kernel_spmd
from concourse.masks import make_identity

dt = mybir.dt
F32, BF16 = dt.float32, dt.bfloat16
AF = mybir.ActivationFunctionType
OP = mybir.AluOpType
BF = ml_dtypes.bfloat16

B = 2
S = 2176
HID = 768
NH = 12
DH = 64
WIN = 17
NCORE = 8
LTOK = 272           # local tokens per batch per core
NTOK = 400           # 272 local + 128 window heads
W2 = 2 * NTOK        # columns of xT: [b0 loc|b0 wh|b1 loc|b1 wh]
GROUPS = [(0, 119), (119, 119), (238, 34)]   # (q0, nq) block groups of 7,7,2
EPS = 1e-5
INNER = 3072

_CACHE = {}
LAST_RESULTS = None


def _body(tc, a):
    nc = tc.nc
    from contextlib import ExitStack
    ctx = ExitStack()
    P = 128

    sb = ctx.enter_context(tc.tile_pool(name="sb", bufs=1))
    sb2 = ctx.enter_context(tc.tile_pool(name="sb2", bufs=2))

    # ---- persistent sbuf ----
    xsb = sb.tile([P, 6 * W2], F32, tag="xsb")          # x^T
    xn = sb.tile([P, 6 * W2], BF16, tag="xn")           # ln1(x)^T bf16
    qt = sb.tile([P, 6 * 544], BF16, tag="qt")          # Q^T
    kt = sb.tile([P, 6 * W2], BF16, tag="kt")           # K^T
    vnat = sb.tile([P, 8 * HID], BF16, tag="vnat")      # V natural per (b,grp/wh)
    attnT = sb.tile([P, 6 * 544], BF16, tag="attnT")
    hid = sb.tile([P, 6 * 544], BF16, tag="hid")
    h2n = sb.tile([P, 6 * 544], BF16, tag="h2n")
    gelu = sb.tile([P, 24 * 544], BF16, tag="gelu")
    maskwh = sb.tile([P, 384], F32, tag="maskwh")
    maskloc = sb.tile([P, 128], F32, tag="maskloc")
    wv = sb.tile([P, 6 * HID], BF16, tag="wv")
    wp = sb.tile([P, 6 * HID], BF16, tag="wp")
    ident = sb.tile([P, P], BF16, tag="ident")
    ones_col = sb.tile([P, 1], BF16, tag="ones_col")
    ones_row = sb.tile([1, P], BF16, tag="ones_row")
    ones_rowf = sb.tile([1, P], F32, tag="ones_rowf")
    bq_sb = sb.tile([P, 18], F32, tag="bq_sb")
    bap_sb = sb.tile([P, 6], F32, tag="bap_sb")
    bfc_sb = sb.tile([P, 24], F32, tag="bfc_sb")
    bmp_sb = sb.tile([P, 6], F32, tag="bmp_sb")
    bv_sb = sb.tile([1, HID], BF16, tag="bv_sb")
    eps_t = sb.tile([1, 1], F32, tag="eps_t")
    rows = sb.tile([1, 4 * W2], F32, tag="rows")        # mu|e2|var|rstd
    rows2 = sb.tile([1, 4 * 544], F32, tag="rows2")
    mu_b = sb.tile([P, W2], F32, tag="mu_b")
    rstd_b = sb.tile([P, W2], F32, tag="rstd_b")
    mu2_b = sb.tile([P, 544], F32, tag="mu2_b")
    rstd2_b = sb.tile([P, 544], F32, tag="rstd2_b")

    make_identity(nc, ident[:, :])
    nc.vector.memset(ones_col[:, :], 1.0)
    nc.vector.memset(ones_row[:, :], 1.0)
    nc.vector.memset(ones_rowf[:, :], 1.0)
    nc.vector.memset(eps_t[:, :], EPS)

    nc.sync.dma_start(xsb[:, :].rearrange("p (k n) -> p k n", k=6),
                      a["xT"].rearrange("(k p) n -> p k n", p=P))
    nc.sync.dma_start(maskwh[:, :], a["maskwh"])
    nc.sync.dma_start(maskloc[:, :], a["maskloc"])
    nc.sync.dma_start(bq_sb[:, :], a["bqkv"].rearrange("(m p) -> p m", p=P))
    nc.sync.dma_start(bap_sb[:, :], a["bap"].rearrange("(m p) -> p m", p=P))
    nc.sync.dma_start(bfc_sb[:, :], a["bfc"].rearrange("(m p) -> p m", p=P))
    nc.sync.dma_start(bmp_sb[:, :], a["bmp"].rearrange("(m p) -> p m", p=P))
    nc.sync.dma_start(bv_sb[:, :], a["bv"].unsqueeze(0))
    nc.gpsimd.dma_start(wv[:, :].rearrange("p (k f) -> p k f", k=6),
                        a["wq"][:, 2 * HID:].rearrange("(k p) f -> p k f", p=P))
    nc.gpsimd.dma_start(wp[:, :].rearrange("p (k f) -> p k f", k=6),
                        a["wp"].rearrange("(k p) f -> p k f", p=P))

    def ln_stats(ps, pst, src, width, rows_t, mu_bt, rstd_bt, sq_from):
        # src: sbuf [128, 6*width] (f32 or bf16); per-column mean/rstd
        half = width // 2
        for h in range(2):
            s_ps = pst.tile([1, half], F32, tag="st")
            ss_ps = pst.tile([1, half], F32, tag="st")
            for k in range(6):
                c0 = width * k + half * h
                if sq_from == "cast":
                    cc = sb2.tile([P, half], BF16, tag="cchunk")
                    nc.scalar.activation(cc[:, :], src[:, c0:c0 + half], AF.Copy)
                else:
                    cc = None
                sqc = sb2.tile([P, half], BF16, tag="sqchunk")
                xin = cc[:, :] if cc is not None else src[:, c0:c0 + half]
                nc.scalar.activation(sqc[:, :], xin, AF.Square)
                nc.tensor.matmul(s_ps[:, :], ones_col[:, 0:1], xin,
                                 start=(k == 0), stop=(k == 5))
                nc.tensor.matmul(ss_ps[:, :], ones_col[:, 0:1], sqc[:, :],
                                 start=(k == 0), stop=(k == 5))
            mu = rows_t[0:1, half * h:half * (h + 1)]
            e2 = rows_t[0:1, width + half * h:width + half * (h + 1)]
            var = rows_t[0:1, 2 * width + half * h:2 * width + half * (h + 1)]
            rstd = rows_t[0:1, 3 * width + half * h:3 * width + half * (h + 1)]
            nc.vector.tensor_scalar_mul(mu, s_ps[:, :], 1.0 / HID)
            nc.vector.tensor_scalar_mul(e2, ss_ps[:, :], 1.0 / HID)
            nc.vector.tensor_tensor(var, mu, mu, OP.mult)
            nc.vector.tensor_tensor(var, e2, var, OP.subtract)
            nc.scalar.activation(var, var, AF.Ln, bias=eps_t[0:1, 0:1])
            nc.scalar.activation(rstd, var, AF.Exp, scale=-0.5)
        for h in range(2):
            for src_off, dstt in ((0, mu_bt), (3 * width, rstd_bt)):
                bc = ps.tile([P, NTOK], F32, tag="pp")
                nc.tensor.matmul(bc[:, 0:half], ones_rowf[0:1, 0:P],
                                 rows_t[0:1, src_off + half * h:src_off + half * (h + 1)],
                                 start=True, stop=True)
                nc.vector.tensor_copy(dstt[:, half * h:half * (h + 1)], bc[:, 0:half])

    def normalize(src, width, mu_bt, rstd_bt, dst):
        for k in range(6):
            c0 = width * k
            tmp = sb2.tile([P, width], F32, tag="lntmp")
            nc.vector.tensor_tensor(tmp[:, :], src[:, c0:c0 + width], mu_bt[:, :],
                                    OP.subtract)
            nc.vector.tensor_tensor(dst[:, c0:c0 + width], tmp[:, :], rstd_bt[:, :],
                                    OP.mult)

    # ================= phase 1: LN1, QKV, V =================
    with tc.tile_pool(name="ps", bufs=2, space="PSUM") as ps, \
         tc.tile_pool(name="pst", bufs=2, space="PSUM") as pst:
        # ---- LN1 ----
        ln_stats(ps, pst, xsb, W2, rows, mu_b, rstd_b, sq_from="cast")
        normalize(xsb, W2, mu_b, rstd_b, xn)

        # ---- QKV projections (Q^T, K^T) ----
        for m in range(12):
            wm = sb2.tile([P, 6 * P], BF16, tag="wtile")
            nc.gpsimd.dma_start(
                wm[:, :].rearrange("p (k f) -> p k f", k=6),
                a["wq"][:, P * m:P * (m + 1)].rearrange("(k p) f -> p k f", p=P))
            for b in range(2):
                n = LTOK if m < 6 else NTOK
                pp = ps.tile([P, NTOK], F32, tag="pp")
                for k in range(6):
                    nc.tensor.matmul(pp[:, 0:n], wm[:, P * k:P * (k + 1)],
                                     xn[:, W2 * k + NTOK * b:W2 * k + NTOK * b + n],
                                     start=(k == 0), stop=(k == 5))
                if m < 6:
                    dstap = qt[:, 544 * m + LTOK * b:544 * m + LTOK * b + n]
                else:
                    dstap = kt[:, W2 * (m - 6) + NTOK * b:W2 * (m - 6) + NTOK * b + n]
                nc.scalar.activation(dstap, pp[:, 0:n], AF.Identity,
                                     bias=bq_sb[:, m:m + 1])

        # ---- V natural ----
        VT = GROUPS + [(LTOK, 128)]
        for b in range(2):
            for t, (t0, ntk) in enumerate(VT):
                for nh in range(2):
                    vp = ps.tile([P, NTOK], F32, tag="pp")
                    for k in range(6):
                        nc.tensor.matmul(
                            vp[0:ntk, 0:384],
                            xn[:, W2 * k + NTOK * b + t0:W2 * k + NTOK * b + t0 + ntk],
                            wv[:, HID * k + 384 * nh:HID * k + 384 * (nh + 1)],
                            start=(k == 0), stop=False)
                    nc.tensor.matmul(vp[0:ntk, 0:384], ones_row[0:1, 0:ntk],
                                     bv_sb[0:1, 384 * nh:384 * (nh + 1)],
                                     start=False, stop=True)
                    nc.vector.tensor_copy(
                        vnat[0:ntk, HID * (4 * b + t) + 384 * nh:
                             HID * (4 * b + t) + 384 * (nh + 1)], vp[0:ntk, 0:384])

    # ================= phase 2: attention =================
    # scores^T = K Q^T  ([keys, queries]); per head pair p (=mm6):
    #   sc bank j (head h=2p+j): wh keys at cols 512j+0:nq via kt row-group
    #   ro=64j (concurrent across j), local keys at cols 512j+128:128+nq.
    #   colsum [1, 2nq] at cols 248:248+2nq of bank 0.
    # av tile (1 bank): av h0 [0:64, 0:nq], av h1 [64:128, 0:nq] (col-group
    #   tiled), recip broadcast [0:128, 128:128+2nq] via K=1 ones matmul.
    with tc.tile_pool(name="scp", bufs=2, space="PSUM") as scp, \
         tc.tile_pool(name="avp", bufs=2, space="PSUM") as avp, \
         tc.tile_pool(name="asb", bufs=3) as asb:
        for b in range(2):
            for t, (q0, nq) in enumerate(GROUPS):
                for p in range(6):
                    n2 = 2 * nq
                    qb = 544 * p + LTOK * b + q0
                    kb = W2 * p + NTOK * b
                    sc = scp.tile([P, 1024], F32, tag="sc")
                    av = avp.tile([P, 512], F32, tag="av")
                    pe1 = asb.tile([P, 256], BF16, tag="pe1")
                    pe2 = asb.tile([P, 256], BF16, tag="pe2")
                    rs = asb.tile([1, 256], BF16, tag="rs")
                    avs = asb.tile([P, 128], BF16, tag="avs")
                    for j in range(2):
                        ro = 64 * j
                        qts = qt[ro:ro + 64, qb:qb + nq]
                        nc.tensor.matmul(
                            sc[0:128, 512 * j:512 * j + nq],
                            kt[ro:ro + 64, kb + LTOK:kb + NTOK], qts,
                            start=True, stop=True, skip_group_check=(j == 1))
                        nc.tensor.matmul(
                            sc[0:nq, 512 * j + 128:512 * j + 128 + nq],
                            kt[ro:ro + 64, kb + q0:kb + q0 + nq], qts,
                            start=True, stop=True, skip_group_check=True)
                    sc3 = sc[:, :].rearrange("q (h n) -> q h n", h=2)
                    nc.vector.tensor_tensor(
                        sc3[0:128, :, 0:nq], sc3[0:128, :, 0:nq],
                        maskwh[0:128, 128 * t:128 * t + nq].unsqueeze(1)
                        .broadcast_to([128, 2, nq]), OP.add)
                    nc.vector.tensor_tensor(
                        sc3[0:nq, :, 128:128 + nq], sc3[0:nq, :, 128:128 + nq],
                        maskloc[0:nq, 0:nq].unsqueeze(1)
                        .broadcast_to([nq, 2, nq]), OP.add)
                    pe1r = pe1[:, :].rearrange("q (h n) -> q h n", h=2)
                    pe2r = pe2[:, :].rearrange("q (h n) -> q h n", h=2)
                    nc.scalar.activation(pe1r[0:128, :, 0:nq],
                                         sc3[0:128, :, 0:nq], AF.Exp, scale=0.125)
                    nc.scalar.activation(pe2r[0:nq, :, 0:nq],
                                         sc3[0:nq, :, 128:128 + nq],
                                         AF.Exp, scale=0.125)
                    nc.tensor.matmul(sc[0:1, 248:248 + n2], ones_col[0:128, 0:1],
                                     pe1r[0:128, :, 0:nq],
                                     start=True, stop=False, skip_group_check=True)
                    nc.tensor.matmul(sc[0:1, 248:248 + n2], ones_col[0:nq, 0:1],
                                     pe2r[0:nq, :, 0:nq],
                                     start=False, stop=True, skip_group_check=True)
                    with nc.allow_low_precision(reason="softmax 1/sum in bf16"):
                        nc.vector.reciprocal(rs[0:1, 0:n2], sc[0:1, 248:248 + n2])
                    nc.tensor.matmul(av[0:128, 128:128 + n2], ones_row[0:1, 0:128],
                                     rs[0:1, 0:n2],
                                     start=True, stop=True, skip_group_check=True)
                    for j in range(2):
                        h = 2 * p + j
                        vwh = HID * (4 * b + 3) + DH * h
                        vlc = HID * (4 * b + t) + DH * h
                        nc.tensor.matmul(av[64 * j:64 * j + 64, 0:nq],
                                         vnat[0:128, vwh:vwh + DH],
                                         pe1[0:128, 128 * j:128 * j + nq],
                                         start=True, stop=False,
                                         skip_group_check=True)
                        nc.tensor.matmul(av[64 * j:64 * j + 64, 0:nq],
                                         vnat[0:nq, vlc:vlc + DH],
                                         pe2[0:nq, 128 * j:128 * j + nq],
                                         start=False, stop=True,
                                         skip_group_check=True)
                    nc.vector.tensor_copy(avs[0:128, 0:nq], av[0:128, 0:nq])
                    for j in range(2):
                        nc.vector.tensor_tensor(
                            attnT[64 * j:64 * j + 64, qb:qb + nq],
                            avs[64 * j:64 * j + 64, 0:nq],
                            av[64 * j:64 * j + 64, 128 + nq * j:128 + nq * (j + 1)],
                            OP.mult)

    # ================= phase 3: attn_proj, LN2, MLP =================
    with tc.tile_pool(name="ps2", bufs=2, space="PSUM") as ps2, \
         tc.tile_pool(name="pst2", bufs=2, space="PSUM") as pst2:
        # ---- attn_proj + residual -> hid (bf16) ----
        for m in range(6):
            for b in range(2):
                pp = ps2.tile([P, NTOK], F32, tag="pp")
                for k in range(6):
                    nc.tensor.matmul(pp[:, 0:LTOK],
                                     wp[:, HID * k + P * m:HID * k + P * (m + 1)],
                                     attnT[:, 544 * k + LTOK * b:544 * k + LTOK * b + LTOK],
                                     start=(k == 0), stop=(k == 5))
                nc.vector.scalar_tensor_tensor(
                    hid[:, 544 * m + LTOK * b:544 * m + LTOK * b + LTOK],
                    pp[:, 0:LTOK], bap_sb[:, m:m + 1],
                    xsb[:, W2 * m + NTOK * b:W2 * m + NTOK * b + LTOK],
                    op0=OP.add, op1=OP.add)

        # ---- LN2 ----
        ln_stats(ps2, pst2, hid, 544, rows2, mu2_b, rstd2_b, sq_from="direct")
        normalize(hid, 544, mu2_b, rstd2_b, h2n)

        # ---- fc + gelu ----
        for m in range(24):
            wm = sb2.tile([P, 6 * P], BF16, tag="wtile")
            nc.gpsimd.dma_start(
                wm[:, :].rearrange("p (k f) -> p k f", k=6),
                a["wf"][:, P * m:P * (m + 1)].rearrange("(k p) f -> p k f", p=P))
            for b in range(2):
                pp = ps2.tile([P, NTOK], F32, tag="pp")
                for k in range(6):
                    nc.tensor.matmul(pp[:, 0:LTOK], wm[:, P * k:P * (k + 1)],
                                     h2n[:, 544 * k + LTOK * b:544 * k + LTOK * b + LTOK],
                                     start=(k == 0), stop=(k == 5))
                nc.scalar.activation(gelu[:, 544 * m + LTOK * b:544 * m + LTOK * b + LTOK],
                                     pp[:, 0:LTOK], AF.Gelu_apprx_tanh,
                                     bias=bfc_sb[:, m:m + 1])

        # ---- mlp_proj + residual -> out ----
        for m in range(6):
            wmc = sb2.tile([P, 24 * P], BF16, tag="wmcol")
            nc.gpsimd.dma_start(
                wmc[:, :].rearrange("p (k f) -> p k f", k=24),
                a["wm"][:, P * m:P * (m + 1)].rearrange("(k p) f -> p k f", p=P))
            for b in range(2):
                pp = ps2.tile([P, NTOK], F32, tag="pp")
                for k in range(24):
                    nc.tensor.matmul(pp[:, 0:LTOK], wmc[:, P * k:P * (k + 1)],
                                     gelu[:, 544 * k + LTOK * b:544 * k + LTOK * b + LTOK],
                                     start=(k == 0), stop=(k == 23))
                ys = sb2.tile([P, LTOK], F32, tag="ys")
                nc.vector.scalar_tensor_tensor(
                    ys[:, :], pp[:, 0:LTOK], bmp_sb[:, m:m + 1],
                    hid[:, 544 * m + LTOK * b:544 * m + LTOK * b + LTOK],
                    op0=OP.add, op1=OP.add)
                nc.sync.dma_start(a["yT"][P * m:P * (m + 1), LTOK * b:LTOK * (b + 1)],
                                  ys[:, :])
    ctx.close()


def _build():
    nc = bacc.Bacc("TRN2", target_bir_lowering=False, debug=False)
    a = {}
    a["xT"] = nc.dram_tensor("xT", [HID, W2], F32, kind="ExternalInput").ap()
    a["maskwh"] = nc.dram_tensor("maskwh", [128, 384], F32, kind="ExternalInput").ap()
    a["maskloc"] = nc.dram_tensor("maskloc", [128, 128], F32, kind="ExternalInput").ap()
    a["wq"] = nc.dram_tensor("wq", [HID, 3 * HID], BF16, kind="ExternalInput").ap()
    a["wp"] = nc.dram_tensor("wp", [HID, HID], BF16, kind="ExternalInput").ap()
    a["wf"] = nc.dram_tensor("wf", [HID, INNER], BF16, kind="ExternalInput").ap()
    a["wm"] = nc.dram_tensor("wm", [INNER, HID], BF16, kind="ExternalInput").ap()
    a["bqkv"] = nc.dram_tensor("bqkv", [3 * HID], F32, kind="ExternalInput").ap()
    a["bv"] = nc.dram_tensor("bv", [HID], BF16, kind="ExternalInput").ap()
    a["bap"] = nc.dram_tensor("bap", [HID], F32, kind="ExternalInput").ap()
    a["bfc"] = nc.dram_tensor("bfc", [INNER], F32, kind="ExternalInput").ap()
    a["bmp"] = nc.dram_tensor("bmp", [HID], F32, kind="ExternalInput").ap()
    a["yT"] = nc.dram_tensor("yT", [HID, 2 * LTOK], F32, kind="ExternalOutput").ap()
    with tile.TileContext(nc) as tc:
        _body(tc, a)
    nc.compile()
    return nc


def _host_prep(inputs):
    x = np.ascontiguousarray(inputs["hidden_states"], np.float32)
    ln1_g = np.asarray(inputs["ln1_g"], np.float32)
    ln1_b = np.asarray(inputs["ln1_b"], np.float32)
    ln2_g = np.asarray(inputs["ln2_g"], np.float32)
    ln2_b = np.asarray(inputs["ln2_b"], np.float32)
    caw = np.asarray(inputs["c_attn_w"], np.float32)
    wq = (caw * ln1_g[:, None]).astype(BF)
    bqkv = (ln1_b @ caw + np.asarray(inputs["c_attn_b"], np.float32)).astype(np.float32)
    wp = np.asarray(inputs["attn_proj_w"], np.float32).astype(BF)
    fcw = np.asarray(inputs["fc_w"], np.float32)
    wf = (fcw * ln2_g[:, None]).astype(BF)
    bfc = (ln2_b @ fcw + np.asarray(inputs["fc_b"], np.float32)).astype(np.float32)
    wm = np.asarray(inputs["mlp_proj_w"], np.float32).astype(BF)
    shared = dict(
        wq=wq, wp=wp, wf=wf, wm=wm, bqkv=bqkv,
        bv=bqkv[2 * HID:].astype(BF),
        bap=np.asarray(inputs["attn_proj_b"], np.float32),
        bfc=bfc, bmp=np.asarray(inputs["mlp_proj_b"], np.float32))

    # local mask (transposed): key i (group-local) visible to query j iff
    # same 17-token window and i <= j; identical for every group/core.
    ii = np.arange(128)
    maskloc = np.where((ii[:, None] // WIN == ii[None, :] // WIN)
                       & (ii[:, None] <= ii[None, :]), 0.0, -1e30
                       ).astype(np.float32)

    wh_idx = np.arange(128) * WIN
    in_maps = []
    for c in range(NCORE):
        t0 = LTOK * c
        cols = []
        for b in range(B):
            cols.append(np.concatenate([x[b, t0:t0 + LTOK], x[b, wh_idx]], 0))
        xT = np.ascontiguousarray(np.concatenate(cols, 0).T)
        # wh mask (transposed): key block k visible to query j of group t
        # iff k < global block of the query.
        maskwh = np.full((128, 384), -1e30, np.float32)
        for t, (q0, nq) in enumerate(GROUPS):
            j = np.arange(nq)
            blk = 16 * c + (q0 + j) // WIN          # [nq]
            maskwh[:, 128 * t:128 * t + nq] = np.where(
                ii[:, None] < blk[None, :], 0.0, -1e30)
        in_maps.append(dict(shared, xT=xT, maskwh=maskwh, maskloc=maskloc))
    return in_maps


def kernel(**inputs):
    global LAST_RESULTS
    if "nc" not in _CACHE:
        _CACHE["nc"] = _build()
    nc = _CACHE["nc"]
    in_maps = _host_prep(inputs)
    res = run_bass_kernel_spmd(nc, in_maps, core_ids=list(range(NCORE)))
    LAST_RESULTS = res
    out = np.zeros((B, S, HID), np.float32)
    for c in range(NCORE):
        yT = res.results[c]["yT"]           # [768, 544]
        t0 = LTOK * c
        out[0, t0:t0 + LTOK] = yT[:, :LTOK].T
        out[1, t0:t0 + LTOK] = yT[:, LTOK:].T
    return out


# revision 4
# speedup vs baseline: 1.2718x; 1.2718x over previous
"""Trainium2 Bass kernel: GPT2 block with ctx-pred sparse attention.

Sharding: tokens (blocks of 17) across 8 cores; window-head tokens
replicated. Each core runs the full block on its 272-token slice per
batch with zero collectives. All activations in transposed layout
[feature(partition), token(free)]; matmuls in bf16 with fp32 accum.

Attention computes transposed scores S^T = K Q^T directly (no PE
transposes), applies multiplicative bf16 masks post-exp, and gets the
softmax denominators broadcast to all partitions with an all-ones
matmul so the reciprocal runs multi-lane. PSUM pools are phase-scoped:
one pool for LN1/QKV/V, one shared pool for attention + the tail so
attn_proj can overlap the second batch's attention.
"""
import numpy as np
import ml_dtypes

import concourse.bass as bass
import concourse.mybir as mybir
import concourse.tile as tile
from concourse import bacc
from concourse.bass_utils import run_bass_kernel_spmd

dt = mybir.dt
F32, BF16 = dt.float32, dt.bfloat16
AF = mybir.ActivationFunctionType
OP = mybir.AluOpType
BF = ml_dtypes.bfloat16

B = 2
S = 2176
HID = 768
NH = 12
DH = 64
WIN = 17
NCORE = 8
LTOK = 272           # local tokens per batch per core
NTOK = 400           # 272 local + 128 window heads
W2 = 2 * NTOK        # columns of xT: [b0 loc|b0 wh|b1 loc|b1 wh]
GROUPS = [(0, 119), (119, 119), (238, 34)]   # (q0, nq) block groups of 7,7,2
EPS = 1e-5
INNER = 3072

_CACHE = {}
LAST_RESULTS = None


def _body(tc, a):
    nc = tc.nc
    from contextlib import ExitStack
    ctx = ExitStack()
    P = 128

    sb = ctx.enter_context(tc.tile_pool(name="sb", bufs=1))
    sb2 = ctx.enter_context(tc.tile_pool(name="sb2", bufs=2))
    wpool = ctx.enter_context(tc.tile_pool(name="wpool", bufs=3))

    # ---- persistent sbuf ----
    xsb = sb.tile([P, 6 * W2], F32, tag="xsb")          # x^T
    xn = sb.tile([P, 6 * W2], BF16, tag="xn")           # ln1(x)^T bf16
    qt = sb.tile([P, 6 * 544], BF16, tag="qt")          # Q^T
    kt = sb.tile([P, 6 * W2], BF16, tag="kt")           # K^T
    vnat = sb.tile([P, 8 * HID], BF16, tag="vnat")      # V natural per (b,grp/wh)
    attnT = sb.tile([P, 6 * 544], BF16, tag="attnT")
    hid = sb.tile([P, 6 * 544], BF16, tag="hid")
    h2n = sb.tile([P, 6 * 544], BF16, tag="h2n")
    gelu = sb.tile([P, 24 * 544], BF16, tag="gelu")
    maskwh = sb.tile([P, 384], BF16, tag="maskwh")      # multiplicative {0,1}
    maskloc = sb.tile([P, 128], BF16, tag="maskloc")
    wv = sb.tile([P, 6 * HID], BF16, tag="wv")
    wp = sb.tile([P, 6 * HID], BF16, tag="wp")
    onesmat = sb.tile([P, P], BF16, tag="onesmat")
    ones_col = sb.tile([P, 1], BF16, tag="ones_col")
    ones_row = sb.tile([1, P], BF16, tag="ones_row")
    ones_rowf = sb.tile([1, P], F32, tag="ones_rowf")
    bq_sb = sb.tile([P, 18], F32, tag="bq_sb")
    bap_sb = sb.tile([P, 6], F32, tag="bap_sb")
    bfc_sb = sb.tile([P, 24], F32, tag="bfc_sb")
    bmp_sb = sb.tile([P, 6], F32, tag="bmp_sb")
    bv_sb = sb.tile([1, HID], BF16, tag="bv_sb")
    eps_t = sb.tile([1, 1], F32, tag="eps_t")
    rows = sb.tile([1, 4 * W2], F32, tag="rows")        # mu|e2|var|rstd
    rows2 = sb.tile([1, 4 * 544], F32, tag="rows2")
    mu_b = sb.tile([P, W2], BF16, tag="mu_b")
    rstd_b = sb.tile([P, W2], BF16, tag="rstd_b")
    mu2_b = sb.tile([P, 544], BF16, tag="mu2_b")
    rstd2_b = sb.tile([P, 544], BF16, tag="rstd2_b")

    nc.vector.memset(onesmat[:, :], 1.0)
    nc.vector.memset(ones_col[:, :], 1.0)
    nc.vector.memset(ones_row[:, :], 1.0)
    nc.vector.memset(ones_rowf[:, :], 1.0)
    nc.vector.memset(eps_t[:, :], EPS)

    # split input load per batch so LN1 stats (h=0 is batch 0) start early
    for b in range(2):
        nc.sync.dma_start(
            xsb[:, :].rearrange("p (k c n) -> p k c n", k=6, c=2)[:, :, b, :],
            a["xT"].rearrange("(k p) (c n) -> p k c n", p=P, c=2)[:, :, b, :])
    nc.sync.dma_start(maskwh[:, :], a["maskwh"])
    nc.sync.dma_start(maskloc[:, :], a["maskloc"])
    nc.sync.dma_start(bq_sb[:, :], a["bqkv"].rearrange("(m p) -> p m", p=P))
    nc.sync.dma_start(bap_sb[:, :], a["bap"].rearrange("(m p) -> p m", p=P))
    nc.sync.dma_start(bfc_sb[:, :], a["bfc"].rearrange("(m p) -> p m", p=P))
    nc.sync.dma_start(bmp_sb[:, :], a["bmp"].rearrange("(m p) -> p m", p=P))
    nc.sync.dma_start(bv_sb[:, :], a["bv"].unsqueeze(0))
    nc.gpsimd.dma_start(wv[:, :].rearrange("p (k f) -> p k f", k=6),
                        a["wq"][:, 2 * HID:].rearrange("(k p) f -> p k f", p=P))
    nc.gpsimd.dma_start(wp[:, :].rearrange("p (k f) -> p k f", k=6),
                        a["wp"].rearrange("(k p) f -> p k f", p=P))

    def ln_stats(pbc, tag_bc, pst, tag_st, src, width, rows_t, mu_bt, rstd_bt,
                 sq_from):
        # src: sbuf [128, 6*width] (f32 or bf16); per-column mean/rstd
        half = width // 2
        for h in range(2):
            s_ps = pst.tile([1, half], F32, tag=tag_st)
            ss_ps = pst.tile([1, half], F32, tag=tag_st)
            for k in range(6):
                c0 = width * k + half * h
                if sq_from == "cast":
                    cc = sb2.tile([P, half], BF16, tag="cchunk")
                    nc.scalar.activation(cc[:, :], src[:, c0:c0 + half], AF.Copy)
                else:
                    cc = None
                sqc = sb2.tile([P, half], BF16, tag="sqchunk")
                xin = cc[:, :] if cc is not None else src[:, c0:c0 + half]
                nc.scalar.activation(sqc[:, :], xin, AF.Square)
                nc.tensor.matmul(s_ps[:, :], ones_col[:, 0:1], xin,
                                 start=(k == 0), stop=(k == 5))
                nc.tensor.matmul(ss_ps[:, :], ones_col[:, 0:1], sqc[:, :],
                                 start=(k == 0), stop=(k == 5))
            mu = rows_t[0:1, half * h:half * (h + 1)]
            e2 = rows_t[0:1, width + half * h:width + half * (h + 1)]
            var = rows_t[0:1, 2 * width + half * h:2 * width + half * (h + 1)]
            rstd = rows_t[0:1, 3 * width + half * h:3 * width + half * (h + 1)]
            nc.vector.tensor_scalar_mul(mu, s_ps[:, :], 1.0 / HID)
            nc.vector.tensor_scalar_mul(e2, ss_ps[:, :], 1.0 / HID)
            nc.vector.tensor_tensor(var, mu, mu, OP.mult)
            nc.vector.tensor_tensor(var, e2, var, OP.subtract)
            nc.scalar.activation(var, var, AF.Ln, bias=eps_t[0:1, 0:1])
            nc.scalar.activation(rstd, var, AF.Exp, scale=-0.5)
        for h in range(2):
            for src_off, dstt in ((0, mu_bt), (3 * width, rstd_bt)):
                bc = pbc.tile([P, NTOK], F32, tag=tag_bc)
                nc.tensor.matmul(bc[:, 0:half], ones_rowf[0:1, 0:P],
                                 rows_t[0:1, src_off + half * h:src_off + half * (h + 1)],
                                 start=True, stop=True)
                nc.vector.tensor_copy(dstt[:, half * h:half * (h + 1)], bc[:, 0:half])

    def normalize(src, width, mu_bt, rstd_bt, dst):
        for k in range(6):
            c0 = width * k
            tmp = sb2.tile([P, width], F32, tag="lntmp")
            nc.vector.tensor_tensor(tmp[:, :], src[:, c0:c0 + width], mu_bt[:, :],
                                    OP.subtract)
            nc.vector.tensor_tensor(dst[:, c0:c0 + width], tmp[:, :], rstd_bt[:, :],
                                    OP.mult)

    # ================= phase 1: LN1, QKV, V =================
    with tc.tile_pool(name="ps", bufs=2, space="PSUM") as ps, \
         tc.tile_pool(name="pst", bufs=2, space="PSUM") as pst:
        # ---- LN1 ----
        ln_stats(ps, "pp", pst, "st", xsb, W2, rows, mu_b, rstd_b,
                 sq_from="cast")
        normalize(xsb, W2, mu_b, rstd_b, xn)

        # ---- QKV projections (Q^T, K^T) ----
        for m in range(12):
            wm = wpool.tile([P, 6 * P], BF16, tag="wtile")
            nc.gpsimd.dma_start(
                wm[:, :].rearrange("p (k f) -> p k f", k=6),
                a["wq"][:, P * m:P * (m + 1)].rearrange("(k p) f -> p k f", p=P))
            for b in range(2):
                n = LTOK if m < 6 else NTOK
                pp = ps.tile([P, NTOK], F32, tag="pp")
                for k in range(6):
                    nc.tensor.matmul(pp[:, 0:n], wm[:, P * k:P * (k + 1)],
                                     xn[:, W2 * k + NTOK * b:W2 * k + NTOK * b + n],
                                     start=(k == 0), stop=(k == 5))
                if m < 6:
                    dstap = qt[:, 544 * m + LTOK * b:544 * m + LTOK * b + n]
                else:
                    dstap = kt[:, W2 * (m - 6) + NTOK * b:W2 * (m - 6) + NTOK * b + n]
                nc.scalar.activation(dstap, pp[:, 0:n], AF.Identity,
                                     bias=bq_sb[:, m:m + 1])

        # ---- V natural ----
        VT = GROUPS + [(LTOK, 128)]
        for b in range(2):
            for t, (t0, ntk) in enumerate(VT):
                for nh in range(2):
                    vp = ps.tile([P, NTOK], F32, tag="pp")
                    for k in range(6):
                        nc.tensor.matmul(
                            vp[0:ntk, 0:384],
                            xn[:, W2 * k + NTOK * b + t0:W2 * k + NTOK * b + t0 + ntk],
                            wv[:, HID * k + 384 * nh:HID * k + 384 * (nh + 1)],
                            start=(k == 0), stop=False)
                    nc.tensor.matmul(vp[0:ntk, 0:384], ones_row[0:1, 0:ntk],
                                     bv_sb[0:1, 384 * nh:384 * (nh + 1)],
                                     start=False, stop=True)
                    nc.vector.tensor_copy(
                        vnat[0:ntk, HID * (4 * b + t) + 384 * nh:
                             HID * (4 * b + t) + 384 * (nh + 1)], vp[0:ntk, 0:384])

    # ========== phase 2+3: attention, attn_proj, LN2, MLP ==========
    # Shared PSUM pool so attn_proj(b0) overlaps attention(b1):
    # sc 2x2 banks + av 2x1 + pp 2x1 = 8 banks.
    with tc.tile_pool(name="scp", bufs=2, space="PSUM") as scp, \
         tc.tile_pool(name="avp", bufs=2, space="PSUM") as avp, \
         tc.tile_pool(name="ps2", bufs=2, space="PSUM") as ps2, \
         tc.tile_pool(name="asb", bufs=3) as asb:

        # ---- attention ----
        # scores^T = K Q^T ([keys, queries]); per head pair p (=mm6):
        # sc bank j (head h=2p+j): wh keys at cols 512j+0:nq (row-group
        # ro=64j, concurrent across j), local keys at cols 512j+128:+nq.
        # pe1/pe2 = masked exp (bf16, [*, nq*j:+nq]); av tile: av h0
        # [0:64, 0:nq], av h1 [64:128, 0:nq] (col-group tiled), sums
        # broadcast to all 128 partitions at [0:128, 128:128+2nq] via
        # all-ones matmul; multi-lane reciprocal -> bcs (SBUF bf16).
        for b in range(2):
            for t, (q0, nq) in enumerate(GROUPS):
                for p in range(6):
                    n2 = 2 * nq
                    qb = 544 * p + LTOK * b + q0
                    kb = W2 * p + NTOK * b
                    sc = scp.tile([P, 1024], F32, tag="sc")
                    av = avp.tile([P, 512], F32, tag="av")
                    pe1 = asb.tile([P, 256], BF16, tag="pe1")
                    pe2 = asb.tile([P, 256], BF16, tag="pe2")
                    bcs = asb.tile([P, 256], BF16, tag="bcs")
                    for j in range(2):
                        ro = 64 * j
                        qts = qt[ro:ro + 64, qb:qb + nq]
                        nc.tensor.matmul(
                            sc[0:128, 512 * j:512 * j + nq],
                            kt[ro:ro + 64, kb + LTOK:kb + NTOK], qts,
                            start=True, stop=True, skip_group_check=(j == 1))
                        nc.tensor.matmul(
                            sc[0:nq, 512 * j + 128:512 * j + 128 + nq],
                            kt[ro:ro + 64, kb + q0:kb + q0 + nq], qts,
                            start=True, stop=True, skip_group_check=True)
                    sc3 = sc[:, :].rearrange("q (h n) -> q h n", h=2)
                    pe1r = pe1[:, 0:n2].rearrange("q (h n) -> q h n", h=2)
                    pe2r = pe2[:, 0:n2].rearrange("q (h n) -> q h n", h=2)
                    nc.scalar.activation(pe1r, sc3[0:128, :, 0:nq],
                                         AF.Exp, scale=0.125)
                    nc.scalar.activation(pe2r[0:nq, :, :],
                                         sc3[0:nq, :, 128:128 + nq],
                                         AF.Exp, scale=0.125)
                    nc.vector.tensor_tensor(
                        pe1r, pe1r,
                        maskwh[0:128, 128 * t:128 * t + nq].unsqueeze(1)
                        .broadcast_to([128, 2, nq]), OP.mult)
                    nc.vector.tensor_tensor(
                        pe2r[0:nq, :, :], pe2r[0:nq, :, :],
                        maskloc[0:nq, 0:nq].unsqueeze(1)
                        .broadcast_to([nq, 2, nq]), OP.mult)
                    nc.tensor.matmul(av[0:128, 128:128 + n2],
                                     onesmat[0:128, 0:128], pe1[0:128, 0:n2],
                                     start=True, stop=False,
                                     skip_group_check=True)
                    nc.tensor.matmul(av[0:128, 128:128 + n2],
                                     onesmat[0:nq, 0:128], pe2[0:nq, 0:n2],
                                     start=False, stop=True,
                                     skip_group_check=True)
                    with nc.allow_low_precision(reason="softmax 1/sum bf16"):
                        nc.vector.reciprocal(bcs[0:128, 0:n2],
                                             av[0:128, 128:128 + n2])
                    for j in range(2):
                        h = 2 * p + j
                        vwh = HID * (4 * b + 3) + DH * h
                        vlc = HID * (4 * b + t) + DH * h
                        nc.tensor.matmul(av[64 * j:64 * j + 64, 0:nq],
                                         vnat[0:128, vwh:vwh + DH],
                                         pe1[0:128, nq * j:nq * (j + 1)],
                                         start=True, stop=False,
                                         skip_group_check=True)
                        nc.tensor.matmul(av[64 * j:64 * j + 64, 0:nq],
                                         vnat[0:nq, vlc:vlc + DH],
                                         pe2[0:nq, nq * j:nq * (j + 1)],
                                         start=False, stop=True,
                                         skip_group_check=True)
                    for j in range(2):
                        nc.vector.tensor_tensor(
                            attnT[64 * j:64 * j + 64, qb:qb + nq],
                            av[64 * j:64 * j + 64, 0:nq],
                            bcs[64 * j:64 * j + 64, nq * j:nq * (j + 1)],
                            OP.mult)

        # ---- attn_proj + residual -> hid (bf16) ----
        for m in range(6):
            for b in range(2):
                pp = ps2.tile([P, NTOK], F32, tag="pp")
                for k in range(6):
                    nc.tensor.matmul(pp[:, 0:LTOK],
                                     wp[:, HID * k + P * m:HID * k + P * (m + 1)],
                                     attnT[:, 544 * k + LTOK * b:544 * k + LTOK * b + LTOK],
                                     start=(k == 0), stop=(k == 5))
                nc.vector.scalar_tensor_tensor(
                    hid[:, 544 * m + LTOK * b:544 * m + LTOK * b + LTOK],
                    pp[:, 0:LTOK], bap_sb[:, m:m + 1],
                    xsb[:, W2 * m + NTOK * b:W2 * m + NTOK * b + LTOK],
                    op0=OP.add, op1=OP.add)

        # ---- LN2 ----
        ln_stats(ps2, "pp", avp, "av", hid, 544, rows2, mu2_b, rstd2_b,
                 sq_from="direct")
        normalize(hid, 544, mu2_b, rstd2_b, h2n)

        # ---- fc + gelu ----
        for m in range(24):
            wm = wpool.tile([P, 6 * P], BF16, tag="wtile")
            nc.gpsimd.dma_start(
                wm[:, :].rearrange("p (k f) -> p k f", k=6),
                a["wf"][:, P * m:P * (m + 1)].rearrange("(k p) f -> p k f", p=P))
            for b in range(2):
                pp = ps2.tile([P, NTOK], F32, tag="pp")
                for k in range(6):
                    nc.tensor.matmul(pp[:, 0:LTOK], wm[:, P * k:P * (k + 1)],
                                     h2n[:, 544 * k + LTOK * b:544 * k + LTOK * b + LTOK],
                                     start=(k == 0), stop=(k == 5))
                nc.scalar.activation(gelu[:, 544 * m + LTOK * b:544 * m + LTOK * b + LTOK],
                                     pp[:, 0:LTOK], AF.Gelu_apprx_tanh,
                                     bias=bfc_sb[:, m:m + 1])

        # ---- mlp_proj + residual -> out ----
        for m in range(6):
            wmc = wpool.tile([P, 24 * P], BF16, tag="wmcol")
            nc.gpsimd.dma_start(
                wmc[:, :].rearrange("p (k f) -> p k f", k=24),
                a["wm"][:, P * m:P * (m + 1)].rearrange("(k p) f -> p k f", p=P))
            for b in range(2):
                pp = ps2.tile([P, NTOK], F32, tag="pp")
                for k in range(24):
                    nc.tensor.matmul(pp[:, 0:LTOK], wmc[:, P * k:P * (k + 1)],
                                     gelu[:, 544 * k + LTOK * b:544 * k + LTOK * b + LTOK],
                                     start=(k == 0), stop=(k == 23))
                ys = sb2.tile([P, LTOK], F32, tag="ys")
                nc.vector.scalar_tensor_tensor(
                    ys[:, :], pp[:, 0:LTOK], bmp_sb[:, m:m + 1],
                    hid[:, 544 * m + LTOK * b:544 * m + LTOK * b + LTOK],
                    op0=OP.add, op1=OP.add)
                nc.sync.dma_start(a["yT"][P * m:P * (m + 1), LTOK * b:LTOK * (b + 1)],
                                  ys[:, :])
    ctx.close()


def _build():
    nc = bacc.Bacc("TRN2", target_bir_lowering=False, debug=False)
    a = {}
    a["xT"] = nc.dram_tensor("xT", [HID, W2], F32, kind="ExternalInput").ap()
    a["maskwh"] = nc.dram_tensor("maskwh", [128, 384], BF16, kind="ExternalInput").ap()
    a["maskloc"] = nc.dram_tensor("maskloc", [128, 128], BF16, kind="ExternalInput").ap()
    a["wq"] = nc.dram_tensor("wq", [HID, 3 * HID], BF16, kind="ExternalInput").ap()
    a["wp"] = nc.dram_tensor("wp", [HID, HID], BF16, kind="ExternalInput").ap()
    a["wf"] = nc.dram_tensor("wf", [HID, INNER], BF16, kind="ExternalInput").ap()
    a["wm"] = nc.dram_tensor("wm", [INNER, HID], BF16, kind="ExternalInput").ap()
    a["bqkv"] = nc.dram_tensor("bqkv", [3 * HID], F32, kind="ExternalInput").ap()
    a["bv"] = nc.dram_tensor("bv", [HID], BF16, kind="ExternalInput").ap()
    a["bap"] = nc.dram_tensor("bap", [HID], F32, kind="ExternalInput").ap()
    a["bfc"] = nc.dram_tensor("bfc", [INNER], F32, kind="ExternalInput").ap()
    a["bmp"] = nc.dram_tensor("bmp", [HID], F32, kind="ExternalInput").ap()
    a["yT"] = nc.dram_tensor("yT", [HID, 2 * LTOK], F32, kind="ExternalOutput").ap()
    with tile.TileContext(nc) as tc:
        _body(tc, a)
    nc.compile()
    return nc


def _host_prep(inputs):
    x = np.ascontiguousarray(inputs["hidden_states"], np.float32)
    ln1_g = np.asarray(inputs["ln1_g"], np.float32)
    ln1_b = np.asarray(inputs["ln1_b"], np.float32)
    ln2_g = np.asarray(inputs["ln2_g"], np.float32)
    ln2_b = np.asarray(inputs["ln2_b"], np.float32)
    caw = np.asarray(inputs["c_attn_w"], np.float32)
    wq = (caw * ln1_g[:, None]).astype(BF)
    bqkv = (ln1_b @ caw + np.asarray(inputs["c_attn_b"], np.float32)).astype(np.float32)
    wp = np.asarray(inputs["attn_proj_w"], np.float32).astype(BF)
    fcw = np.asarray(inputs["fc_w"], np.float32)
    wf = (fcw * ln2_g[:, None]).astype(BF)
    bfc = (ln2_b @ fcw + np.asarray(inputs["fc_b"], np.float32)).astype(np.float32)
    wm = np.asarray(inputs["mlp_proj_w"], np.float32).astype(BF)
    shared = dict(
        wq=wq, wp=wp, wf=wf, wm=wm, bqkv=bqkv,
        bv=bqkv[2 * HID:].astype(BF),
        bap=np.asarray(inputs["attn_proj_b"], np.float32),
        bfc=bfc, bmp=np.asarray(inputs["mlp_proj_b"], np.float32))

    # local mask (transposed, multiplicative): key i visible to query j
    # iff same 17-token window and i <= j; identical for every group/core.
    ii = np.arange(128)
    maskloc = (((ii[:, None] // WIN) == (ii[None, :] // WIN))
               & (ii[:, None] <= ii[None, :])).astype(BF)

    wh_idx = np.arange(128) * WIN
    in_maps = []
    for c in range(NCORE):
        t0 = LTOK * c
        cols = []
        for b in range(B):
            cols.append(np.concatenate([x[b, t0:t0 + LTOK], x[b, wh_idx]], 0))
        xT = np.ascontiguousarray(np.concatenate(cols, 0).T)
        # wh mask (transposed, multiplicative): key block k visible to
        # query j of group t iff k < global block of the query.
        maskwh = np.zeros((128, 384), BF)
        for t, (q0, nq) in enumerate(GROUPS):
            j = np.arange(nq)
            blk = 16 * c + (q0 + j) // WIN          # [nq]
            maskwh[:, 128 * t:128 * t + nq] = (
                ii[:, None] < blk[None, :]).astype(BF)
        in_maps.append(dict(shared, xT=xT, maskwh=maskwh, maskloc=maskloc))
    return in_maps


def kernel(**inputs):
    global LAST_RESULTS
    if "nc" not in _CACHE:
        _CACHE["nc"] = _build()
    nc = _CACHE["nc"]
    in_maps = _host_prep(inputs)
    res = run_bass_kernel_spmd(nc, in_maps, core_ids=list(range(NCORE)))
    LAST_RESULTS = res
    out = np.zeros((B, S, HID), np.float32)
    for c in range(NCORE):
        yT = res.results[c]["yT"]           # [768, 544]
        t0 = LTOK * c
        out[0, t0:t0 + LTOK] = yT[:, :LTOK].T
        out[1, t0:t0 + LTOK] = yT[:, LTOK:].T
    return out
